# revision 1
# baseline (speedup 1.0000x reference)
"""Llama GQA attention (b=2, s=2048, h=4096, 32 Q heads / 8 KV heads, rope)
as a Bass/Tile kernel for 8 Trainium2 NeuronCores.

Sharding: data-parallel over batch (2) x tensor-parallel over heads (4).
Core c = (b, r), b = c // 4, r = c % 4 handles batch b with Q heads
[8r, 8r+8) and KV heads [2r, 2r+2).  Wq/Wk/Wv column-sharded, Wo
row-sharded; per-core output is a partial sum over the TP group which the
host reduces (fp32 adds).

On-core dataflow (all activations feature-major, i.e. transposed):
  XT [H, T] is DMA'd into SBUF ONCE and stays resident through phase 1;
  all projection matmuls stream it from SBUF (no HBM refetch).
  QT/KT [heads*128, T] with RoPE fused at PSUM eviction; V [T, 256]
  token-major, its transposing matmuls fused into the single-Q-head jobs
  so their ldweights hide under the Q matmul streams.
  Attention runs as a software pipeline over (q-tile, head) jobs:
  S^T[k,q] = KT-tile.T @ QT (PE) + causal mask (DVE) + exp (ACT) of job N
  interleave with AV matmuls of job N-1, so ACT never gates the PE.
  Row sums via DVE accumulation of the exp tiles plus ONE ones-vector
  matmul per job (not per tile); normalization applied at O^T eviction
  through a PE outer-product broadcast of 1/sum.
  O^T tiles feed the Wo projection producing OUT^T [H, T] which the host
  transposes / reduces.
"""

import math
import sys

import numpy as np

for _p in ("/opt/trn_rl_repo",):
    if _p not in sys.path:
        sys.path.insert(0, _p)

import ml_dtypes  # noqa: E402

import concourse.bass as bass  # noqa: E402
import concourse.mybir as mybir  # noqa: E402
import concourse.tile as tile  # noqa: E402
from concourse.alu_op_type import AluOpType  # noqa: E402

F32 = mybir.dt.float32
BF16 = mybir.dt.bfloat16
AF = mybir.ActivationFunctionType

# full problem constants
B, S, H = 2, 2048, 4096
NUM_HEADS, NUM_KV_HEADS, D = 32, 8, 128
ROPE_THETA = 10000.0
TP, DP = 4, 2
MASK_VAL = -30000.0


def build_nc(T=S, HID=H, NQL=NUM_HEADS // TP, NKVL=NUM_KV_HEADS // TP,
             HOUT=H, TQ=512):
    """One-core SPMD program.  T tokens, HID hidden, NQL local Q heads,
    NKVL local KV heads, HOUT output features, TQ q-tile width."""
    assert T % TQ == 0 and TQ % 128 == 0 and HID % 128 == 0
    GRP = NQL // NKVL            # q heads per kv head
    QC = NQL * D                 # local q columns
    KC = NKVL * D                # local kv columns
    KX = HID // 128              # contraction chunks for projections
    NTOK = T // TQ               # token tiles of width TQ
    NT128 = T // 128             # token tiles of width 128
    NKT = TQ // 128              # 128-wide k tiles per q tile
    MW = TQ + (TQ - 128)         # additive causal mask width

    nc = bass.Bass()
    xt = nc.dram_tensor("xt", [HID, T], BF16, kind="ExternalInput")
    wq = nc.dram_tensor("wq", [HID, QC], BF16, kind="ExternalInput")
    wk = nc.dram_tensor("wk", [HID, KC], BF16, kind="ExternalInput")
    wv = nc.dram_tensor("wv", [HID, KC], BF16, kind="ExternalInput")
    wo = nc.dram_tensor("wo", [QC, HOUT], BF16, kind="ExternalInput")
    cosb = nc.dram_tensor("cosb", [128, T], BF16, kind="ExternalInput")
    sinb = nc.dram_tensor("sinb", [128, T], BF16, kind="ExternalInput")  # sign-folded
    maskb = nc.dram_tensor("maskb", [128, MW], BF16, kind="ExternalInput")
    idmb = nc.dram_tensor("idmb", [128, 128], BF16, kind="ExternalInput")
    outp = nc.dram_tensor("outp", [HOUT, T], BF16, kind="ExternalOutput")

    inv_sqrt_d = 1.0 / math.sqrt(D)

    with tile.TileContext(nc) as tc:
        with (
            tc.tile_pool(name="resident", bufs=1) as res,
            tc.tile_pool(name="const", bufs=1) as const,
        ):
            # resident SBUF arrays (live across all phases)
            qt = [res.tile([128, T], BF16, tag=f"qt{h}", name=f"qt{h}") for h in range(NQL)]
            kt = [res.tile([128, T], BF16, tag=f"kt{h}", name=f"kt{h}") for h in range(NKVL)]
            vt = [res.tile([128, KC], BF16, tag=f"v{t}", name=f"v{t}") for t in range(NT128)]
            cos_sb = res.tile([128, T], BF16, tag="cos", name="cos_sb")
            sin_sb = res.tile([128, T], BF16, tag="sin", name="sin_sb")
            mask_sb = res.tile([128, MW], BF16, tag="mask", name="mask_sb")
            idm = const.tile([128, 128], BF16, tag="idm", name="idm")
            ones_col = const.tile([128, 1], BF16)
            ones_row = const.tile([1, 128], BF16)
            nc.vector.memset(ones_col[:], 1.0)
            nc.vector.memset(ones_row[:], 1.0)

            # ---------------- phase 1: projections ----------------
            # XT resident in SBUF for the whole phase: fetched from HBM once.
            with (
                tc.tile_pool(name="xts", bufs=1) as xpool,
                tc.tile_pool(name="wt", bufs=8) as wpool,
                tc.tile_pool(name="rope_tmp", bufs=2) as rpool,
                tc.tile_pool(name="pj_psum", bufs=1, space="PSUM") as pp,
            ):
                xtsb = [xpool.tile([128, T], BF16, tag=f"x{k}", name=f"x{k}")
                        for k in range(KX)]

                def rope_evict(ps, dst_ap, tok0):
                    """dst = ps*cos + rot_half(ps)*sin  (sin sign-folded)."""
                    cw = cos_sb[:, tok0:tok0 + TQ]
                    sw = sin_sb[:, tok0:tok0 + TQ]
                    r = rpool.tile([128, TQ], F32, tag="rot", name="rot")
                    nc.scalar.copy(r[0:64, :], ps[64:128, :])
                    nc.scalar.copy(r[64:128, :], ps[0:64, :])
                    t1 = rpool.tile([128, TQ], F32, tag="t1", name="t1")
                    nc.vector.tensor_tensor(t1[:], ps[:], cw, op=AluOpType.mult)
                    nc.gpsimd.tensor_tensor(r[:], r[:], sw, op=AluOpType.mult)
                    nc.gpsimd.tensor_tensor(dst_ap, t1[:], r[:], op=AluOpType.add)

                # Jobs: (wsrc, dst, h, vgroup, tagbase).  All single-head;
                # K heads first, then Q heads, the last four fused with V
                # quarter-sweeps (V ldweights hide under the Q matmul streams).
                # Non-fused jobs alternate psum tag sets 0-3 / 4-7 so a job's
                # first matmul never waits on the previous job's rope eviction.
                VG = 4
                jobs = [(wk, kt, 0, None, 0),
                        (wq, qt, 0, None, 0), (wq, qt, 1, None, 4),
                        (wq, qt, 2, None, 0), (wq, qt, 3, None, 4)]
                for i in range(4):
                    jobs.append((wq, qt, 4 + i, i * VG, 0))

                for job_i, (wsrc, dst, h0, vg0, tb) in enumerate(jobs):
                    nheads = 2 if job_i == 0 else 1
                    ps = [pp.tile([128, TQ], F32, tag=f"pp{tb + j}",
                                  name=f"pj{j}")
                          for j in range(NTOK * nheads)]
                    if vg0 is not None:
                        psv = [pp.tile([128, TQ], F32, tag=f"pp{4 + i}",
                                       name=f"pv{i}")
                               for i in range(VG)]
                    for k in range(KX):
                        wt_sb = wpool.tile([128, nheads * 128], BF16, tag="w", name="wt_sb")
                        nc.sync.dma_start(
                            out=wt_sb[:],
                            in_=wsrc[k * 128:(k + 1) * 128,
                                     h0 * 128:(h0 + nheads) * 128])
                        if vg0 is not None:
                            wv_sb = wpool.tile([128, KC], BF16, tag="wv", name="wv_sb")
                            nc.sync.dma_start(out=wv_sb[:],
                                              in_=wv[k * 128:(k + 1) * 128, :])
                        if job_i == 0:
                            nsp = 4 if k < 8 else (2 if k < 16 else 1)
                            for c in range(nsp):
                                cs = c * (T // nsp)
                                ce = (c + 1) * (T // nsp)
                                nc.sync.dma_start(
                                    out=xtsb[k][:, cs:ce],
                                    in_=xt[k * 128:(k + 1) * 128, cs:ce])
                            if k == 8:
                                nc.sync.dma_start(out=cos_sb[:], in_=cosb[:])
                                nc.sync.dma_start(out=sin_sb[:], in_=sinb[:])
                            if k == 16:
                                nc.sync.dma_start(out=mask_sb[:], in_=maskb[:])
                                nc.sync.dma_start(out=idm[:], in_=idmb[:])
                        for i in range(nheads):
                            for j in range(NTOK):
                                nc.tensor.matmul(
                                    ps[i * NTOK + j][:],
                                    lhsT=wt_sb[:, i * 128:(i + 1) * 128],
                                    rhs=xtsb[k][:, j * TQ:(j + 1) * TQ],
                                    start=(k == 0), stop=(k == KX - 1))
                        if vg0 is not None:
                            for i in range(VG):
                                t0 = (vg0 + i) * 128
                                nc.tensor.matmul(
                                    psv[i][:, 0:KC],
                                    lhsT=xtsb[k][:, t0:t0 + 128],
                                    rhs=wv_sb[:],
                                    start=(k == 0), stop=(k == KX - 1))
                    for i in range(nheads):
                        for j in range(NTOK):
                            rope_evict(ps[i * NTOK + j],
                                       dst[h0 + i][:, j * TQ:(j + 1) * TQ],
                                       j * TQ)
                    if vg0 is not None:
                        for i in range(VG):
                            nc.vector.tensor_copy(vt[vg0 + i][:], psv[i][:, 0:KC])

            # ---------------- phases 2+3 ----------------
            with tc.tile_pool(name="post", bufs=1) as post:
                ot = [post.tile([128, T], BF16, tag=f"ot{h}", name=f"ot{h}")
                      for h in range(NQL)]

                # phase 2: attention, software-pipelined over (qi, h) jobs.
                # Job N's S^T matmuls + mask + exp + DVE row-sum accumulation
                # interleave with job N-1's AV matmuls; one ones-col matmul
                # per job turns the accumulated exp-sum into row sums, and the
                # 1/sum broadcast-normalize runs off the PE's critical path.
                with (
                    tc.tile_pool(name="es", bufs=3) as epool,
                    tc.tile_pool(name="at_small", bufs=2) as spool,
                    tc.tile_pool(name="ps_s", bufs=4, space="PSUM") as psum_s,
                    tc.tile_pool(name="ps_o", bufs=2, space="PSUM") as psum_o,
                    tc.tile_pool(name="ps_r", bufs=1, space="PSUM") as psum_r,
                    tc.tile_pool(name="ps_b", bufs=1, space="PSUM") as psum_b,
                ):
                    def emit_s_tile(job, ki):
                        off = max(0, ki * 128 - job["q0"])  # first valid q col
                        w = TQ - off
                        ps_s = psum_s.tile([128, TQ], F32, tag="s", name="ps_s")
                        masked = ki >= job["nk"] - NKT
                        pe_mask = masked and job["qi"] == 0
                        nc.tensor.matmul(
                            ps_s[:, off:TQ],
                            lhsT=kt[job["kvh"]][:, ki * 128:(ki + 1) * 128],
                            rhs=qt[job["h"]][:, job["q0"] + off:job["q0"] + TQ],
                            start=True, stop=not pe_mask)
                        if pe_mask:
                            nc.tensor.matmul(
                                ps_s[:, off:TQ], lhsT=idm[:],
                                rhs=mask_sb[:, (TQ - 128):(TQ - 128) + w],
                                start=False, stop=True)
                        elif masked:
                            mv = mask_sb[:, (TQ - 128):(TQ - 128) + w]
                            nc.vector.tensor_tensor(ps_s[:, off:TQ],
                                                    ps_s[:, off:TQ], mv,
                                                    op=AluOpType.add)
                        e = epool.tile([128, TQ], BF16, tag=f"e{ki}",
                                       name=f"e{ki}")
                        nc.scalar.activation(e[:, off:TQ], ps_s[:, off:TQ],
                                             AF.Exp, scale=inv_sqrt_d)
                        job["es"].append((e, off))

                    def emit_av_tile(job, ki):
                        if ki == 0:
                            job["ps_o"] = psum_o.tile([128, TQ], F32, tag="o", name="ps_o")
                        e, off = job["es"][ki]
                        nc.tensor.matmul(
                            job["ps_o"][:, off:TQ],
                            lhsT=vt[ki][:, job["kvh"] * D:(job["kvh"] + 1) * D],
                            rhs=e[:, off:TQ],
                            start=(ki == 0), stop=(ki == job["nk"] - 1))

                    def emit_sums(job):
                        # row sums of exp: consecutive ones-col matmuls so the
                        # stationary operand is loaded once for the whole chain
                        ps_row = psum_r.tile([1, TQ], F32, tag="row", name="ps_row")
                        for ki in range(job["nk"]):
                            e, off = job["es"][ki]
                            nc.tensor.matmul(ps_row[:, off:TQ],
                                             lhsT=ones_col[:],
                                             rhs=e[:, off:TQ],
                                             start=(ki == 0),
                                             stop=(ki == job["nk"] - 1))
                        job["ps_row"] = ps_row

                    def emit_norm(job):
                        rsb = spool.tile([1, TQ], BF16, tag="rsb", name="rsb")
                        if job["qi"] == 0:
                            nc.vector.tensor_copy(rsb[:], job["ps_row"][:])
                        else:
                            nc.scalar.copy(rsb[:], job["ps_row"][:])
                        ps_bc = psum_b.tile([128, TQ], F32, tag="b", name="ps_bc")
                        nc.tensor.matmul(ps_bc[:], lhsT=ones_row[:],
                                         rhs=rsb[:], start=True, stop=True)
                        bcr = spool.tile([128, TQ], F32, tag="bcr", name="bcr")
                        if job["qi"] <= 1:
                            lns = spool.tile([128, TQ], F32, tag="lns", name="lns")
                            nc.scalar.activation(lns[:], ps_bc[:], AF.Ln)
                            nc.scalar.activation(bcr[:], lns[:], AF.Exp,
                                                 scale=-1.0)
                        else:
                            nc.vector.reciprocal(bcr[:], ps_bc[:])
                        nc.vector.tensor_tensor(
                            ot[job["h"]][:, job["q0"]:job["q0"] + TQ],
                            job["ps_o"][:], bcr[:], op=AluOpType.mult)

                    prev = None
                    for qi in range(NTOK):
                        for h in range(NQL):
                            cur = {"qi": qi, "h": h, "kvh": h // GRP,
                                   "nk": (qi + 1) * NKT, "q0": qi * TQ,
                                   "es": []}
                            nk_p = prev["nk"] if prev else 0
                            for ki in range(max(cur["nk"], nk_p)):
                                if ki < cur["nk"]:
                                    emit_s_tile(cur, ki)
                                if prev is not None and ki < nk_p:
                                    emit_av_tile(prev, ki)
                            if prev is not None:
                                emit_sums(prev)
                                emit_norm(prev)
                            prev = cur
                    for ki in range(prev["nk"]):
                        emit_av_tile(prev, ki)
                    emit_sums(prev)
                    emit_norm(prev)

                # ---------------- phase 3: output projection ----------------
                CT = QC // 128  # contraction chunks (== NQL)
                with (
                    tc.tile_pool(name="wo_sb", bufs=2) as wopool,
                    tc.tile_pool(name="ob", bufs=4) as obpool,
                    tc.tile_pool(name="po_psum", bufs=2, space="PSUM") as pop,
                ):
                    NG = 4  # n-tiles per weight fetch group
                    for ng in range(0, HOUT // 128, NG):
                        gn = min(NG, HOUT // 128 - ng)
                        wos = []
                        for c in range(CT):
                            w = wopool.tile([128, gn * 128], BF16,
                                            tag=f"wo{c}", name=f"wosb{c}")
                            nc.sync.dma_start(
                                out=w[:], in_=wo[c * 128:(c + 1) * 128,
                                                ng * 128:(ng + gn) * 128])
                            wos.append(w)
                        for i in range(gn):
                            ni = ng + i
                            ps = [pop.tile([128, TQ], F32, tag=f"po{j}",
                                           name=f"po{j}")
                                  for j in range(NTOK)]
                            for c in range(CT):
                                for j in range(NTOK):
                                    nc.tensor.matmul(
                                        ps[j][:],
                                        lhsT=wos[c][:, i * 128:(i + 1) * 128],
                                        rhs=ot[c][:, j * TQ:(j + 1) * TQ],
                                        start=(c == 0), stop=(c == CT - 1))
                            for j in range(NTOK):
                                ob = obpool.tile([128, TQ], BF16, tag="ob", name="ob")
                                nc.scalar.copy(ob[:], ps[j][:])
                                nc.sync.dma_start(
                                    out=outp[ni * 128:(ni + 1) * 128,
                                             j * TQ:(j + 1) * TQ],
                                    in_=ob[:])
    legalize_wait_counts(nc)
    return nc


def legalize_wait_counts(nc):
    """walrus DIRECT2D descriptors accept a single sync-wait; Tile can emit
    more (data wait + queue-head wait).  Hoist excess waits onto
    EventSemaphore instructions inserted just before, on the same engine."""
    n_new = 0
    for f in nc.m.functions:
        for blk in f.blocks:
            idx = 0
            insts = blk.instructions
            while idx < len(insts):
                inst = insts[idx]
                si = getattr(inst, "sync_info", None)
                cap = 2 if isinstance(inst, mybir.InstEventSemaphore) else 1
                waits = list(si.on_wait) if si is not None and si.on_wait else []
                if len(waits) > cap:
                    keep, extra = waits[-cap:], waits[:-cap]
                    si.on_wait = keep
                    for i in range(0, len(extra), 2):
                        ev = mybir.InstEventSemaphore(
                            name=f"waitsplit_{n_new}", ins=[], outs=[])
                        n_new += 1
                        ev.engine = inst.engine
                        ev.sync_info = mybir.SyncInfo(
                            on_wait=extra[i:i + 2], on_update=[])
                        nc.register_instruction(ev)
                        insts.insert(idx, ev)
                        idx += 1
                idx += 1
    return n_new


def _host_inputs(hidden_states, position_ids, Wq, Wk, Wv, Wo):
    """Build the 8 per-core input maps."""
    hs = np.asarray(hidden_states, dtype=np.float32)
    pos = np.asarray(position_ids)
    Wq = np.asarray(Wq, dtype=np.float32)
    Wk = np.asarray(Wk, dtype=np.float32)
    Wv = np.asarray(Wv, dtype=np.float32)
    Wo = np.asarray(Wo, dtype=np.float32)
    b, s, h = hs.shape
    qc = h // TP
    kc = (NUM_KV_HEADS * D) // TP
    bf = ml_dtypes.bfloat16

    # rope tables per batch, feature-major, sin sign-folded for rotate_half
    inv_freq = 1.0 / (ROPE_THETA ** (np.arange(0, D, 2, dtype=np.float32) / D))
    maps = []
    TQ = 512
    mw = TQ + (TQ - 128)
    i_idx = np.arange(128)[:, None]
    m_idx = np.arange(mw)[None, :]
    maskb = np.where(m_idx >= i_idx + (TQ - 128), 0.0, MASK_VAL).astype(bf)

    for c in range(DP * TP):
        bb, r = c // TP, c % TP
        t = pos[bb].astype(np.float64)  # [s]
        ang = t[None, :] * np.concatenate([inv_freq, inv_freq])[:, None]  # [128, s]
        cosb = np.cos(ang).astype(bf)
        sinb = np.sin(ang)
        sinb[0:64, :] *= -1.0  # rotate_half sign fold
        sinb = sinb.astype(bf)
        maps.append({
            "idmb": np.eye(128, dtype=bf),
            "xt": np.ascontiguousarray(hs[bb].T).astype(bf),
            "wq": np.ascontiguousarray(Wq[:, r * qc:(r + 1) * qc]).astype(bf),
            "wk": np.ascontiguousarray(Wk[:, r * kc:(r + 1) * kc]).astype(bf),
            "wv": np.ascontiguousarray(Wv[:, r * kc:(r + 1) * kc]).astype(bf),
            "wo": np.ascontiguousarray(Wo[r * qc:(r + 1) * qc, :]).astype(bf),
            "cosb": cosb,
            "sinb": sinb,
            "maskb": maskb,
        })
    return maps


_NC_CACHE = {}


def _get_nc():
    if "nc" not in _NC_CACHE:
        _NC_CACHE["nc"] = build_nc()
    return _NC_CACHE["nc"]


def kernel(hidden_states, position_ids, Wq, Wk, Wv, Wo, _results_hook=None):
    from concourse.bass_utils import run_bass_kernel_spmd

    maps = _host_inputs(hidden_states, position_ids, Wq, Wk, Wv, Wo)
    nc = _get_nc()
    res = run_bass_kernel_spmd(nc, maps, list(range(DP * TP)))
    if _results_hook is not None:
        _results_hook(res)
    b, s, h = np.asarray(hidden_states).shape
    out = np.zeros((b, s, h), dtype=np.float32)
    for c in range(DP * TP):
        bb = c // TP
        out[bb] += np.asarray(res.results[c]["outp"], dtype=np.float32).T
    return out


if __name__ == "__main__":
    # smoke: build the full-size program and print instruction counts
    nc = build_nc()
    print("built ok")



# revision 13
# speedup vs baseline: 1.0817x; 1.0817x over previous
"""Llama GQA attention (b=2, s=2048, h=4096, 32 Q heads / 8 KV heads, rope)
as a Bass/Tile kernel for 8 Trainium2 NeuronCores.

Sharding: data-parallel over batch (2) x tensor-parallel over heads (4).
Core c = (b, r), b = c // 4, r = c % 4 handles batch b with Q heads
[8r, 8r+8) and KV heads [2r, 2r+2).  Wq/Wk/Wv column-sharded, Wo
row-sharded; per-core output is a partial sum over the TP group which the
host reduces (fp32 adds).

v2 layout (single merged program per core):
  Phase 1 (projections): XT [H,T] streamed once into SBUF (2 DMA queues),
  host-pre-tiled weights fetched in contiguous quarter-slices on the
  scalar queue.  Jobs: K(2 heads, 8 psum banks) then 8 single-Q-head jobs
  on alternating 4-bank sets; the last 4 fuse the V-transposing matmuls
  (V token-pairs packed 2-per-psum-bank).  RoPE fused at PSUM eviction
  entirely on DVE+GPSIMD via cross-partition-base tensor_tensor (no ACT
  copies); eviction order tuned so the next job's first bank is free.

  Phase 2+3 (attention + output projection, merged): per (q-tile, head)
  job: S^T = KT.T @ QT with a narrow 128-wide additive causal mask folded
  into the PE accumulation, exp on ACT, AV matmuls lagged TWO jobs behind
  so they never wait on exp.  Row sums via a per-job DVE chain over the
  exp tiles plus ONE ones-vector matmul; 1/sum computed on the [1,TQ] row
  BEFORE a PE broadcast; normalization applied at O^T eviction.  Wo
  projection tasks (one 128-row output tile x TQ tokens each, weights
  prefetched quarter-wise) are interleaved between attention jobs as PE
  filler as soon as their q-tile row completes, so ACT/DVE latency never
  idles the PE and the phases share one pipeline.
"""

import math
import sys

import numpy as np

for _p in ("/opt/trn_rl_repo",):
    if _p not in sys.path:
        sys.path.insert(0, _p)

import ml_dtypes  # noqa: E402

import concourse.bass as bass  # noqa: E402
import concourse.mybir as mybir  # noqa: E402
import concourse.tile as tile  # noqa: E402
from concourse.alu_op_type import AluOpType  # noqa: E402

F32 = mybir.dt.float32
BF16 = mybir.dt.bfloat16
AF = mybir.ActivationFunctionType

# full problem constants
B, S, H = 2, 2048, 4096
NUM_HEADS, NUM_KV_HEADS, D = 32, 8, 128
ROPE_THETA = 10000.0
TP, DP = 4, 2
MASK_VAL = -30000.0


def build_nc(T=S, HID=H, NQL=NUM_HEADS // TP, NKVL=NUM_KV_HEADS // TP,
             HOUT=H, TQ=512, debug=False):
    """One-core SPMD program.  T tokens, HID hidden, NQL local Q heads,
    NKVL local KV heads, HOUT output features, TQ q-tile width."""
    assert T % TQ == 0 and TQ % 128 == 0 and HID % 128 == 0
    GRP = NQL // NKVL            # q heads per kv head
    QC = NQL * D                 # local q columns
    KC = NKVL * D                # local kv columns
    KX = HID // 128              # contraction chunks for projections
    NTOK = T // TQ               # token tiles of width TQ
    NT128 = T // 128             # token tiles of width 128
    NKT = TQ // 128              # 128-wide k tiles per q tile
    VG = 4                       # v token tiles per fused job
    NV = NT128 // VG             # number of fused (Q+V) jobs
    CT = QC // 128               # phase-3 contraction chunks (== NQL)
    NT_OUT = HOUT // 128         # phase-3 output tiles
    NL = min(8, NT_OUT)          # n-tiles per wo quarter
    NQD = NT_OUT // NL           # wo quarters
    WQTR = max(1, KX // 4)       # weight chunks per fetch piece

    nc = bass.Bass()
    xt = nc.dram_tensor("xt", [HID, T], BF16, kind="ExternalInput")
    # K heads then Q heads, each [KX*128] k-major tiled columns
    wqk = nc.dram_tensor("wqk", [128, (NKVL + NQL) * KX * 128], BF16,
                         kind="ExternalInput")
    wvt = nc.dram_tensor("wvt", [128, KX * KC], BF16, kind="ExternalInput")
    # [qd][c][nl][128] tiling of the wo row-slice
    wot = nc.dram_tensor("wot", [128, NQD * CT * NL * 128], BF16,
                         kind="ExternalInput")
    cossin = nc.dram_tensor("cossin", [128, T], BF16, kind="ExternalInput")
    maskb = nc.dram_tensor("maskb", [128, 128], BF16, kind="ExternalInput")
    idmb = nc.dram_tensor("idmb", [128, 128], BF16, kind="ExternalInput")
    outp = nc.dram_tensor("outp", [HOUT, T], BF16, kind="ExternalOutput")
    if debug:
        qdbg = nc.dram_tensor("qdbg", [NQL * 128, T], BF16, kind="ExternalOutput")
        kdbg = nc.dram_tensor("kdbg", [NKVL * 128, T], BF16, kind="ExternalOutput")
        vdbg = nc.dram_tensor("vdbg", [(NT128 // 2) * 128, 2 * KC], BF16, kind="ExternalOutput")
        odbg = nc.dram_tensor("odbg", [NQL * 128, T], BF16, kind="ExternalOutput")

    inv_sqrt_d = 1.0 / math.sqrt(D)

    with tile.TileContext(nc) as tc:
        with (
            tc.tile_pool(name="resident", bufs=1) as res,
            tc.tile_pool(name="const", bufs=1) as const,
            tc.tile_pool(name="pb", bufs=1, space="PSUM") as pb,
        ):
            # resident SBUF (live across both phases)
            qt = [res.tile([128, T], BF16, tag=f"qt{h}", name=f"qt{h}")
                  for h in range(NQL)]
            kt = [res.tile([128, T], BF16, tag=f"kt{h}", name=f"kt{h}")
                  for h in range(NKVL)]
            # v token-pair tiles: [:, 0:KC] = tokens 2t, [:, KC:2KC] = 2t+1
            vtp = [res.tile([128, 2 * KC], BF16, tag=f"v{t}", name=f"v{t}")
                   for t in range(NT128 // 2)]
            idm = const.tile([128, 128], BF16, tag="idm", name="idm")
            mask_sb = const.tile([128, 128], BF16, tag="mask", name="mask_sb")
            ones_col = const.tile([128, 1], BF16)
            ones_row = const.tile([1, 128], BF16)
            nc.vector.memset(ones_col[:], 1.0)
            nc.vector.memset(ones_row[:], 1.0)

            def ps_bank(bk, name):
                return pb.tile([128, TQ], F32, tag=f"b{bk}", name=name)

            # ---------------- phase 1: projections ----------------
            with (
                tc.tile_pool(name="xts", bufs=1) as xpool,
                tc.tile_pool(name="wt", bufs=2) as wpool,
                tc.tile_pool(name="rope_tmp", bufs=1) as rpool,
            ):
                xtsb = [xpool.tile([128, T], BF16, tag=f"x{k}", name=f"x{k}")
                        for k in range(KX)]
                cs = xpool.tile([128, T], BF16, tag="cs", name="cs")
                # cos rows 0:64 (duplicated across halves), sin rows 64:128
                nc.scalar.dma_start(out=cs[:], in_=cossin[:])
                nc.scalar.dma_start(out=mask_sb[:], in_=maskb[:])
                nc.scalar.dma_start(out=idm[:], in_=idmb[:])

                # X stream: even chunks on sync queue, odd on gpsimd queue;
                # first chunks split for a fast start.
                for k in range(KX):
                    eng = nc.sync if (k % 2 == 0) else nc.gpsimd
                    nsp = 4 if k < 2 else (2 if k < 6 else 1)
                    for c in range(nsp):
                        cs0 = c * (T // nsp)
                        ce0 = (c + 1) * (T // nsp)
                        eng.dma_start(out=xtsb[k][:, cs0:ce0],
                                      in_=xt[k * 128:(k + 1) * 128, cs0:ce0])

                def rope_evict(ps, dst_ap, tok0, tmp_i):
                    """dst = ps*cos + rotate_half(ps)*sin, via half-tile
                    cross-partition tensor_tensor ops on DVE/GPSIMD."""
                    c64 = cs[0:64, tok0:tok0 + TQ]
                    s64 = cs[64:128, tok0:tok0 + TQ]
                    t1 = rpool.tile([128, TQ], F32, tag=f"t1{tmp_i}",
                                    name="t1")
                    r = rpool.tile([128, TQ], BF16, tag=f"r{tmp_i}", name="r")
                    # GPSIMD cannot read PSUM: DVE does the two PSUM-reading
                    # multiplies, GPSIMD the SBUF-only combine.
                    nc.vector.tensor_tensor(t1[0:64, :], ps[0:64, :], c64,
                                            op=AluOpType.mult)
                    nc.vector.tensor_tensor(t1[64:128, :], ps[64:128, :], c64,
                                            op=AluOpType.mult)
                    nc.vector.tensor_tensor(r[0:64, :], ps[64:128, :], s64,
                                            op=AluOpType.mult)
                    nc.vector.tensor_tensor(r[64:128, :], ps[0:64, :], s64,
                                            op=AluOpType.mult)
                    nc.gpsimd.tensor_tensor(dst_ap[0:64, :], t1[0:64, :],
                                            r[0:64, :],
                                            op=AluOpType.subtract)
                    nc.gpsimd.tensor_tensor(dst_ap[64:128, :], t1[64:128, :],
                                            r[64:128, :], op=AluOpType.add)

                # job list: (kind, qhead, qbanks, vg0, vbanks)
                setA = list(range(NTOK))
                setB = list(range(NTOK, 2 * NTOK))
                jobs = [("K", None, (setA + setB)[:NKVL * NTOK], None, None)]
                nfirst = NQL - NV  # plain Q jobs
                for i in range(NQL):
                    qb = setA if i % 2 == 0 else setB
                    if i < nfirst:
                        jobs.append(("Q", i, qb, None, None))
                    else:
                        vb = ([setB[-2], setB[-1]] if i % 2 == 0
                              else [setA[-2], setA[-1]])
                        vg0 = (i - nfirst) * VG
                        jobs.append(("QV", i, qb, vg0, vb))

                for job_i, (kind, qh, qbanks, vg0, vbanks) in enumerate(jobs):
                    nheads = NKVL if kind == "K" else 1
                    wbase = 0 if kind == "K" else (NKVL + qh) * KX * 128
                    ps = [ps_bank(qbanks[i * NTOK + j], f"pj{i}_{j}")
                          for i in range(nheads) for j in range(NTOK)]
                    if vg0 is not None:
                        psv = [ps_bank(vbanks[i], f"pv{i}")
                               for i in range(VG // 2)]
                    # weight fetch: quarter pieces, tag per head parity
                    NP = KX // WQTR  # pieces per head
                    wtl = {}
                    for p in range(NP):
                        for i in range(nheads):
                            w = wpool.tile([128, WQTR * 128], BF16,
                                           tag=f"wh{i % 2}", name=f"w{i}_{p}")
                            nc.scalar.dma_start(
                                out=w[:],
                                in_=wqk[:, wbase + (i * KX + p * WQTR) * 128:
                                        wbase + (i * KX + (p + 1) * WQTR) * 128])
                            wtl[(i, p)] = w
                    if vg0 is not None:
                        wv_sb = {}
                        for p in range(NP):
                            w = wpool.tile([128, WQTR * KC], BF16, tag="wv",
                                           name=f"wv{p}")
                            nc.scalar.dma_start(
                                out=w[:], in_=wvt[:, p * WQTR * KC:
                                                  (p + 1) * WQTR * KC])
                            wv_sb[p] = w
                    for k in range(KX):
                        p, ko = k // WQTR, k % WQTR
                        for i in range(nheads):
                            wsl = wtl[(i, p)][:, ko * 128:(ko + 1) * 128]
                            for j in range(NTOK):
                                nc.tensor.matmul(
                                    ps[i * NTOK + j][:],
                                    lhsT=wsl,
                                    rhs=xtsb[k][:, j * TQ:(j + 1) * TQ],
                                    start=(k == 0), stop=(k == KX - 1))
                        if vg0 is not None:
                            vsl = wv_sb[p][:, ko * KC:(ko + 1) * KC]
                            for i in range(VG):
                                t0 = (vg0 + i) * 128
                                # two token-tiles pack one psum bank as ONE
                                # accumulation group: start marks the whole
                                # 2KB zero-region pending-zero, so only the
                                # bank's first write may carry start=True
                                nc.tensor.matmul(
                                    psv[i // 2][:, (i % 2) * KC:
                                                (i % 2) * KC + KC],
                                    lhsT=xtsb[k][:, t0:t0 + 128],
                                    rhs=vsl,
                                    start=(k == 0 and i % 2 == 0),
                                    stop=(k == KX - 1 and i % 2 == 1))
                    # evictions: V pairs first (fast copies free the banks the
                    # next fused job's V matmuls need), then rope.
                    if vg0 is not None:
                        for i in range(VG // 2):
                            nc.vector.tensor_copy(vtp[vg0 // 2 + i][:],
                                                  psv[i][:, 0:2 * KC])
                    ev = list(range(nheads * NTOK))
                    if kind == "QV" and job_i != len(jobs) - 1 and NTOK >= 2:
                        # free the tail banks (next fused job's psv) first
                        ev = ev[NTOK - 2:] + ev[:NTOK - 2]
                    for e_i, ij in enumerate(ev):
                        i, j = ij // NTOK, ij % NTOK
                        dst = (kt[i] if kind == "K" else qt[qh])
                        rope_evict(ps[ij], dst[:, j * TQ:(j + 1) * TQ],
                                   j * TQ, e_i % 2)

            # ---------------- phases 2+3 (merged) ----------------
            with (
                tc.tile_pool(name="post", bufs=1) as post,
                tc.tile_pool(name="es", bufs=3) as epool,
                tc.tile_pool(name="small", bufs=2) as spool,
                tc.tile_pool(name="wo_sb", bufs=2) as wopool,
                tc.tile_pool(name="ob", bufs=4) as obpool,
            ):
                ot = [post.tile([128, T], BF16, tag=f"ot{h}", name=f"ot{h}")
                      for h in range(CT)]

                PS_S = [2, 3]          # S^T tiles (ping-pong)
                PS_O = [4, 5]          # O^T accumulators
                PS_R = 6               # row-sum bank (rows rotate 0/32/64)
                PS_3 = [0, 1, 7]       # phase-3 + broadcast accumulators
                row_bank = ps_bank(PS_R, "rowbank")

                s_rot = [0]
                o_rot = [0]
                p3_rot = [0]

                def emit_s_tile(job, ki):
                    off = max(0, ki * 128 - job["q0"])
                    ps_s = ps_bank(PS_S[s_rot[0] % 2], "ps_s")
                    s_rot[0] += 1
                    masked = ki >= job["nk"] - NKT
                    nc.tensor.matmul(
                        ps_s[:, off:TQ],
                        lhsT=kt[job["kvh"]][:, ki * 128:(ki + 1) * 128],
                        rhs=qt[job["h"]][:, job["q0"] + off:job["q0"] + TQ],
                        start=True, stop=not masked)
                    if masked:
                        nc.tensor.matmul(
                            ps_s[:, off:off + 128], lhsT=idm[:],
                            rhs=mask_sb[:], start=False, stop=True)
                    e = epool.tile([128, TQ], BF16, tag=f"e{ki}",
                                   name=f"e{ki}")
                    nc.scalar.activation(e[:, off:TQ], ps_s[:, off:TQ],
                                         AF.Exp, scale=inv_sqrt_d)
                    job["es"].append((e, off))
                    # esum chain on DVE (bf16); each tile valid on [off:TQ].
                    # Chain depth <= nk (16) of positive values in (0,1]:
                    # ~0.2% RMS on the softmax denominator, well in budget.
                    if ki == 0:
                        esum = spool.tile([128, TQ], BF16, tag="esum",
                                          name="esum")
                        job["esum"] = esum
                        nc.vector.tensor_copy(esum[:], e[:])
                    else:
                        esum = job["esum"]
                        with nc.allow_low_precision(
                                reason="bf16 softmax-denominator chain"):
                            nc.vector.tensor_tensor(esum[:, off:TQ],
                                                    esum[:, off:TQ],
                                                    e[:, off:TQ],
                                                    op=AluOpType.add)

                def emit_av_tile(job, ki):
                    if ki == 0:
                        job["ps_o"] = ps_bank(PS_O[o_rot[0] % 2], "ps_o")
                        o_rot[0] += 1
                    e, off = job["es"][ki]
                    tp, half = ki // 2, ki % 2
                    nc.tensor.matmul(
                        job["ps_o"][:, off:TQ],
                        lhsT=vtp[tp][:, half * KC + job["kvh"] * D:
                                     half * KC + (job["kvh"] + 1) * D],
                        rhs=e[:, off:TQ],
                        start=(ki == 0), stop=(ki == job["nk"] - 1))

                def emit_sums(job):
                    # single ones-col matmul over the accumulated exp sums
                    r = job["slot"] % 3
                    rs = row_bank[32 * r:32 * r + 1, :]
                    nc.tensor.matmul(rs[:], lhsT=ones_col[:],
                                     rhs=job["esum"][:],
                                     start=True, stop=True)
                    rsb = spool.tile([1, TQ], BF16, tag="rsb", name="rsb")
                    with nc.allow_low_precision(
                            reason="bf16 1/rowsum feeds bf16 broadcast"):
                        nc.vector.reciprocal(rsb[:], rs[:])
                    job["rsb"] = rsb

                def emit_norm(job):
                    ps_bc = ps_bank(PS_3[p3_rot[0] % 3], "ps_bc")
                    p3_rot[0] += 1
                    nc.tensor.matmul(ps_bc[:], lhsT=ones_row[:],
                                     rhs=job["rsb"][:], start=True, stop=True)
                    bcr = spool.tile([128, TQ], BF16, tag="bcr", name="bcr")
                    nc.scalar.copy(bcr[:], ps_bc[:])
                    nc.vector.tensor_tensor(
                        ot[job["h"]][:, job["q0"]:job["q0"] + TQ],
                        job["ps_o"][:], bcr[:], op=AluOpType.mult)

                # ---- phase-3 task machinery ----
                # wo quarters stream through 2 rotating buffers on the (idle)
                # gpsimd DMA queue; quarters are refetched per q-tile j.
                wo_live = []  # [(qd, tile)], oldest first, max 2

                def get_wo(qd):
                    for q, t in wo_live:
                        if q == qd:
                            return t
                    w = wopool.tile([128, CT * NL * 128], BF16, tag="wo",
                                    name=f"wo{qd}")
                    nc.gpsimd.dma_start(
                        out=w[:], in_=wot[:, qd * CT * NL * 128:
                                          (qd + 1) * CT * NL * 128])
                    wo_live.append((qd, w))
                    if len(wo_live) > 2:
                        wo_live.pop(0)
                    return w

                get_wo(0)

                def emit_p3(task):
                    qd, j, nl = task
                    wsb = get_wo(qd)
                    if nl == 0 and NQD > 1:
                        get_wo((qd + 1) % NQD)  # prefetch the next quarter
                    ps3 = ps_bank(PS_3[p3_rot[0] % 3], "ps3")
                    p3_rot[0] += 1
                    for c in range(CT):
                        nc.tensor.matmul(
                            ps3[:],
                            lhsT=wsb[:, (c * NL + nl) * 128:
                                     (c * NL + nl) * 128 + 128],
                            rhs=ot[c][:, j * TQ:(j + 1) * TQ],
                            start=(c == 0), stop=(c == CT - 1))
                    ob = obpool.tile([128, TQ], BF16, tag="ob", name="ob")
                    nc.scalar.copy(ob[:], ps3[:])
                    nt = qd * NL + nl
                    nc.sync.dma_start(
                        out=outp[nt * 128:(nt + 1) * 128,
                                 j * TQ:(j + 1) * TQ],
                        in_=ob[:])

                # j-major, quarters sequential within each j (quarters are
                # refetched per j; 2-buffer rotation handles the streaming)
                p3_seq = [(qd, j, nl) for j in range(NTOK)
                          for qd in range(NQD) for nl in range(NL)]
                p3_ptr = [0]
                rows_done = [-1]

                def pop_p3():
                    if (p3_ptr[0] < len(p3_seq)
                            and p3_seq[p3_ptr[0]][1] <= rows_done[0]):
                        t = p3_seq[p3_ptr[0]]
                        p3_ptr[0] += 1
                        return t
                    return None

                # ---- merged pipeline ----
                jl = []
                for qi in range(NTOK):
                    for h in range(NQL):
                        jl.append({"qi": qi, "h": h, "kvh": h // GRP,
                                   "nk": (qi + 1) * NKT, "q0": qi * TQ,
                                   "es": [], "slot": len(jl)})

                P3_PER_SLOT = 3
                for idx in range(len(jl) + 2):
                    J = jl[idx] if idx < len(jl) else None
                    Jm1 = jl[idx - 1] if 1 <= idx <= len(jl) else None
                    Jm2 = jl[idx - 2] if idx >= 2 else None
                    nk_j = J["nk"] if J else 0
                    nk_p = Jm2["nk"] if Jm2 else 0
                    for ki in range(max(nk_j, nk_p)):
                        if J and ki < nk_j:
                            emit_s_tile(J, ki)
                        if Jm2 and ki < nk_p:
                            emit_av_tile(Jm2, ki)
                    if Jm1 is not None:
                        emit_sums(Jm1)
                    if Jm2 is not None:
                        emit_norm(Jm2)
                        if Jm2["h"] == NQL - 1:
                            rows_done[0] = Jm2["qi"]
                    for _ in range(P3_PER_SLOT):
                        t = pop_p3()
                        if t is None:
                            break
                        emit_p3(t)
                # drain remaining phase-3 tasks
                while True:
                    t = pop_p3()
                    if t is None:
                        break
                    emit_p3(t)
                if debug:
                    for h in range(NQL):
                        nc.sync.dma_start(out=qdbg[h * 128:(h + 1) * 128, :], in_=qt[h][:])
                        nc.sync.dma_start(out=odbg[h * 128:(h + 1) * 128, :], in_=ot[h][:])
                    for h in range(NKVL):
                        nc.sync.dma_start(out=kdbg[h * 128:(h + 1) * 128, :], in_=kt[h][:])
                    for t2 in range(NT128 // 2):
                        nc.sync.dma_start(out=vdbg[t2 * 128:(t2 + 1) * 128, :], in_=vtp[t2][:])

    legalize_wait_counts(nc)
    return nc


def legalize_wait_counts(nc):
    """walrus DIRECT2D descriptors accept a single sync-wait; Tile can emit
    more (data wait + queue-head wait).  Hoist excess waits onto
    EventSemaphore instructions inserted just before, on the same engine."""
    n_new = 0
    for f in nc.m.functions:
        for blk in f.blocks:
            idx = 0
            insts = blk.instructions
            while idx < len(insts):
                inst = insts[idx]
                si = getattr(inst, "sync_info", None)
                cap = 2 if isinstance(inst, mybir.InstEventSemaphore) else 1
                waits = list(si.on_wait) if si is not None and si.on_wait else []
                if len(waits) > cap:
                    keep, extra = waits[-cap:], waits[:-cap]
                    si.on_wait = keep
                    for i in range(0, len(extra), 2):
                        ev = mybir.InstEventSemaphore(
                            name=f"waitsplit_{n_new}", ins=[], outs=[])
                        n_new += 1
                        ev.engine = inst.engine
                        ev.sync_info = mybir.SyncInfo(
                            on_wait=extra[i:i + 2], on_update=[])
                        nc.register_instruction(ev)
                        insts.insert(idx, ev)
                        idx += 1
                idx += 1
    return n_new


def _host_inputs(hidden_states, position_ids, Wq, Wk, Wv, Wo):
    """Build the 8 per-core input maps (weights pre-tiled for the kernel)."""
    hs = np.asarray(hidden_states, dtype=np.float32)
    pos = np.asarray(position_ids)
    Wq = np.asarray(Wq, dtype=np.float32)
    Wk = np.asarray(Wk, dtype=np.float32)
    Wv = np.asarray(Wv, dtype=np.float32)
    Wo = np.asarray(Wo, dtype=np.float32)
    b, s, h = hs.shape
    NQL = NUM_HEADS // TP
    NKVL = NUM_KV_HEADS // TP
    qc = NQL * D
    kc = NKVL * D
    KX = h // 128
    bf = ml_dtypes.bfloat16

    CT = qc // 128
    NT_OUT = h // 128
    NL = min(8, NT_OUT)
    NQD = NT_OUT // NL

    inv_freq = 1.0 / (ROPE_THETA ** (np.arange(0, D, 2, dtype=np.float32) / D))
    i_idx = np.arange(128)[:, None]
    j_idx = np.arange(128)[None, :]
    maskb = np.where(j_idx < i_idx, MASK_VAL, 0.0).astype(bf)

    def tile_w(w):
        # [h, cols] -> [128, nheads * KX * 128] head-major, k-chunk-tiled
        cols = w.shape[1]
        nh = cols // 128
        # out[p, (hd*KX + k)*128 + c] = w[k*128+p, hd*128+c]
        wr = w.reshape(KX, 128, nh, 128)            # k, p, hd, c
        wr = wr.transpose(1, 2, 0, 3)               # p, hd, k, c
        return np.ascontiguousarray(wr.reshape(128, nh * KX * 128)).astype(bf)

    def tile_wv(w):
        # out[p, k*kc + c] = w[k*128+p, c]
        wr = w.reshape(KX, 128, kc).transpose(1, 0, 2)
        return np.ascontiguousarray(wr.reshape(128, KX * kc)).astype(bf)

    def tile_wo(w):
        # w: [qc, h].  out[p, ((qd*CT + cc)*NL + nl)*128 + col]
        #            = w[cc*128+p, (qd*NL+nl)*128+col]
        wr = w.reshape(CT, 128, NQD, NL, 128)       # cc, p, qd, nl, col
        wr = wr.transpose(1, 2, 0, 3, 4)            # p, qd, cc, nl, col
        return np.ascontiguousarray(
            wr.reshape(128, NQD * CT * NL * 128)).astype(bf)

    maps = []
    for c in range(DP * TP):
        bb, r = c // TP, c % TP
        t = pos[bb].astype(np.float64)  # [s]
        ang = t[None, :] * inv_freq[:, None]          # [64, s]
        cossin = np.empty((128, s), dtype=np.float64)
        cossin[0:64] = np.cos(ang)
        cossin[64:128] = np.sin(ang)
        wq_s = Wq[:, r * qc:(r + 1) * qc]
        wk_s = Wk[:, r * kc:(r + 1) * kc]
        maps.append({
            "idmb": np.eye(128, dtype=bf),
            "xt": np.ascontiguousarray(hs[bb].T).astype(bf),
            "wqk": np.concatenate([tile_w(wk_s), tile_w(wq_s)], axis=1),
            "wvt": tile_wv(Wv[:, r * kc:(r + 1) * kc]),
            "wot": tile_wo(Wo[r * qc:(r + 1) * qc, :]),
            "cossin": cossin.astype(bf),
            "maskb": maskb,
        })
    return maps


_NC_CACHE = {}


def _get_nc():
    if "nc" not in _NC_CACHE:
        _NC_CACHE["nc"] = build_nc()
    return _NC_CACHE["nc"]


def kernel(hidden_states, position_ids, Wq, Wk, Wv, Wo, _results_hook=None):
    from concourse.bass_utils import run_bass_kernel_spmd

    maps = _host_inputs(hidden_states, position_ids, Wq, Wk, Wv, Wo)
    nc = _get_nc()
    res = run_bass_kernel_spmd(nc, maps, list(range(DP * TP)))
    if _results_hook is not None:
        _results_hook(res)
    b, s, h = np.asarray(hidden_states).shape
    out = np.zeros((b, s, h), dtype=np.float32)
    for c in range(DP * TP):
        bb = c // TP
        out[bb] += np.asarray(res.results[c]["outp"], dtype=np.float32).T
    return out


if __name__ == "__main__":
    nc = build_nc()
    print("built ok")
